# revision 65
# baseline (speedup 1.0000x reference)
"""Distributed attention kernel for 8 TRN2 NeuronCores.

Problem: B=2, N=M=4096, C=512, H=8 heads, D=64.
  q/k/v = linear(query/key/value); attn = softmax(q k^T / sqrt(D)); out = proj(attn v) + bias

Sharding: 1 head per core (tensor parallel over heads); each core runs both
batches for its head. After each batch's attention, an 8-core AllToAll swaps
head-shards for token-shards and every core runs the full output projection
for that batch (batch 0's A2A + projection hide under batch 1's attention).
Each core emits out0 (a batch-0 token-slice) and out1 (a batch-1 slice);
the host keeps out0 from cores 0-3 and out1 from cores 4-7.

Layouts (per core, head h = core index):
  inputs staged transposed+bf16 on host: queryT/keyT/valueT (C, N)
  qT, kT: (D=64, N) = WxT.T @ xT            (PE, bf16)
  v:      (M, D) natural + a ones column     (so P@V also yields softmax denom)
  sT:     (m-tile 128, n-chunk 1024) = k q^T (PSUM f32, two 512 matmuls)
  P:      exp(sT / sqrt(D))                  (one wide ACT op; no max-sub:
                                              scores ~ N(0,1), max ~6)
  oT:     (65, 512) += v_aug.T @ P-half      (row 64 = sum_m P = softmax denom)
  xT:     oT[0:64] * (1/denom)               (denom bcast via ones-matmul, f32r)
  A2A_b:  shard j = xT_b[:, 1024*(j%4):...]  (bf16, 1MB/rank, all 8 cores)
  proj:   out_b(c', n) = WpT.T @ xfull + bp  (bias per-partition on ACT)
"""

import numpy as np
import ml_dtypes

import concourse.bass as bass
import concourse.mybir as mybir
import concourse.tile as tile
from concourse import bacc
from concourse.bass import ds, ts
from concourse.bass_utils import run_bass_kernel_spmd

BF16 = mybir.dt.bfloat16
F32 = mybir.dt.float32
F32R = mybir.dt.float32r
FP8 = mybir.dt.float8e4
DR = mybir.MatmulPerfMode.DoubleRow

B, N, M, C, H, D = 2, 4096, 4096, 512, 8, 64
SCALE = D ** -0.5
NCORES = 8
NSLICE = N // 4          # 1024 tokens per core after A2A
WCH = 1024               # scores/exp chunk width
EXP_FUNC = mybir.ActivationFunctionType.Exp
ID_FUNC = mybir.ActivationFunctionType.Identity
I16 = mybir.dt.int16
# DVE takes the top XDVE columns of each 1024-wide exp via a bf16 Schraudolph:
# bf16bits(exp(s/8)) ~= round(s * (0.125*128*log2 e) + (127*128 + 0.75)).
# The sawtooth error (~3% on P) mostly cancels in the softmax ratio; the
# uniform-scale part cancels exactly.
XDVE = 256
XACT = WCH - XDVE
SCHRA_A = float(0.125 * 128 * np.log2(np.e))
SCHRA_B = float(127 * 128 + 0.75)


def build_trivial_nc():
    """Same I/O signature, near-zero work — for differential wall-clock timing."""
    nc = bacc.Bacc("TRN2", target_bir_lowering=False, debug=False, num_devices=NCORES)
    params = []
    for b in range(B):
        params.append(nc.declare_dram_parameter(f"queryT{b}", [C, N], BF16, isOutput=False))
        params.append(nc.declare_dram_parameter(f"keyT{b}", [C, M], BF16, isOutput=False))
        params.append(nc.declare_dram_parameter(f"valueT{b}", [C, M], BF16, isOutput=False))
    nc.declare_dram_parameter("wqT", [C, D], BF16, isOutput=False)
    nc.declare_dram_parameter("wkT", [C, D], BF16, isOutput=False)
    nc.declare_dram_parameter("wvT", [C, D], BF16, isOutput=False)
    nc.declare_dram_parameter("wpT", [C, C], BF16, isOutput=False)
    nc.declare_dram_parameter("bp", [C], F32, isOutput=False)
    outs = [nc.declare_dram_parameter(f"out{b}", [C, NSLICE], F32, isOutput=True) for b in range(B)]
    with tile.TileContext(nc) as tc:
        with tc.tile_pool(name="sb", bufs=1) as sb:
            t = sb.tile([128, 16], F32, name="t")
            nc.sync.dma_start(out=t, in_=params[0][:].bitcast(F32)[0:128, 0:16])
            for b in range(B):
                nc.sync.dma_start(out=outs[b][0:128, 0:16], in_=t)
    nc.finalize()
    return nc


def build_nc(taps=False, fp8=False):
    nc = bacc.Bacc(
        "TRN2", target_bir_lowering=False, debug=False, num_devices=NCORES
    )
    tap = {}
    if taps:
        qk_dt = FP8 if fp8 else BF16
        tap["qT"] = nc.declare_dram_parameter("tap_qT", [D, N], qk_dt, isOutput=True)
        tap["kT"] = nc.declare_dram_parameter("tap_kT", [D, M], qk_dt, isOutput=True)
        tap["v"] = nc.declare_dram_parameter("tap_v", [128, 32, D + 1], BF16, isOutput=True)
        tap["xT"] = nc.declare_dram_parameter("tap_xT", [D, B * N], BF16, isOutput=True)

    # Per-core DRAM parameters. Transposed activations are identical on every
    # core; weight slices are per-head.
    qryT = [nc.declare_dram_parameter(f"queryT{b}", [C, N], BF16, isOutput=False) for b in range(B)]
    keyT = [nc.declare_dram_parameter(f"keyT{b}", [C, M], BF16, isOutput=False) for b in range(B)]
    valT = [nc.declare_dram_parameter(f"valueT{b}", [C, M], BF16, isOutput=False) for b in range(B)]
    wqT = nc.declare_dram_parameter("wqT", [C, D], BF16, isOutput=False)
    wkT = nc.declare_dram_parameter("wkT", [C, D], BF16, isOutput=False)
    wvT = nc.declare_dram_parameter("wvT", [C, D], BF16, isOutput=False)
    wpT = nc.declare_dram_parameter("wpT", [C, C], BF16, isOutput=False)
    bp = nc.declare_dram_parameter("bp", [C], F32, isOutput=False)
    vones = nc.declare_dram_parameter("vones", [128, 32], BF16, isOutput=False)
    onesr = nc.declare_dram_parameter("onesr", [1, D], F32R, isOutput=False)
    out_ext = [nc.declare_dram_parameter(f"out{b}", [C, NSLICE], F32, isOutput=True) for b in range(B)]

    with tile.TileContext(nc) as tc:
        with (
            tc.tile_pool(name="consts", bufs=1) as consts,
            tc.tile_pool(name="inputs", bufs=1) as inputs,
            tc.tile_pool(name="qk", bufs=2) as qkpool,
            tc.tile_pool(name="vp", bufs=2) as vpool,
            tc.tile_pool(name="ppool", bufs=3) as ppool,
            tc.tile_pool(name="small", bufs=2) as small,
            tc.tile_pool(name="xt", bufs=1) as xtpool,
            tc.tile_pool(name="psS", bufs=2, space="PSUM") as psS,   # scores, 2 banks/slot
            tc.tile_pool(name="psO", bufs=1, space="PSUM") as psO,   # attention accumulators
            tc.tile_pool(name="psB", bufs=2, space="PSUM") as psB,   # qkv/proj/bcast matmuls
            tc.tile_pool(name="dram", bufs=1, space="DRAM") as dram,
        ):
            # ---- constants / weights ----
            wq_sb = consts.tile([128, 4, D], BF16, name="wq_sb")
            wk_sb = consts.tile([128, 4, D], BF16, name="wk_sb")
            wv_sb = consts.tile([128, 4, D], BF16, name="wv_sb")
            wp_sb = consts.tile([128, 4, C], BF16, name="wp_sb")
            bp_sb = consts.tile([128, 4], F32, name="bp_sb")
            ones_r = consts.tile([1, D], F32R, name="ones_r")
            nc.sync.dma_start(out=ones_r, in_=onesr[:])
            nc.sync.dma_start(out=wq_sb, in_=wqT[:].rearrange("(t p) d -> p t d", p=128))
            nc.sync.dma_start(out=wk_sb, in_=wkT[:].rearrange("(t p) d -> p t d", p=128))
            nc.sync.dma_start(out=wv_sb, in_=wvT[:].rearrange("(t p) d -> p t d", p=128))
            nc.sync.dma_start(out=wp_sb, in_=wpT[:].rearrange("(t p) d -> p t d", p=128))
            nc.sync.dma_start(out=bp_sb, in_=bp[:].rearrange("(t p) -> p t", p=128))

            a2a_in = [dram.tile([NCORES, D, NSLICE], BF16, name=f"a2a_in{b}") for b in range(B)]
            a2a_out = [dram.tile([NCORES, D, NSLICE], BF16, name=f"a2a_out{b}") for b in range(B)]

            def once(fn):
                done = [False]
                def wrapper():
                    if not done[0]:
                        done[0] = True
                        fn()
                return wrapper

            def qkv_units(b, defer_loads=False):
                """Allocate tiles; return (loads, k/q/v unit closures).

                Each unit closure emits one PSUM-sized piece of the q/k/v
                projections. The caller weaves them into the attention loop so
                the in-order PE stream never sits on a long block of
                projection matmuls while ACT is idle.
                """
                qry_sb = inputs.tile([128, 4, N], BF16, name="qry_sb")
                key_sb = inputs.tile([128, 4, M], BF16, name="key_sb")
                val_sb = inputs.tile([128, 4, M], BF16, name="val_sb")
                if fp8:
                    qT_tmp = qkpool.tile([D, N], FP8, name="qT_tmp", tag="qT_tmp")
                    kT_tmp = qkpool.tile([D, M], FP8, name="kT_tmp", tag="kT_tmp")
                    # DoubleRow-packed: [p, s, n] = row (s*32+p); q and k use
                    # the same packing so the contraction order is consistent.
                    qT_sb = qkpool.tile([32, 2, N], FP8, name="qT_sb", tag="qT_sb")
                    kT_sb = qkpool.tile([32, 2, M], FP8, name="kT_sb", tag="kT_sb")
                else:
                    qT_tmp = qT_sb = qkpool.tile([D, N], BF16, name="qT_sb", tag="qT_sb")
                    kT_tmp = kT_sb = qkpool.tile([D, M], BF16, name="kT_sb", tag="kT_sb")
                v_sb = vpool.tile([128, 32, D + 1], BF16, name="v_sb")

                @once
                def loads():
                    # column-chunked so the first kT/qT units can start after
                    # ~0.5MB instead of a full 4MB tensor load
                    with nc.named_scope(f"qkv{b}"):
                        def chunk(dst, src, j):
                            nc.sync.dma_start(
                                out=dst[:, :, ds(j * 512, 512)],
                                in_=src[:, ds(j * 512, 512)].rearrange(
                                    "(t p) n -> p t n", p=128
                                ),
                            )
                        chunk(key_sb, keyT[b], 0)
                        chunk(qry_sb, qryT[b], 0)
                        chunk(qry_sb, qryT[b], 1)
                        for j in range(1, 8):
                            chunk(key_sb, keyT[b], j)
                        nc.sync.dma_start(out=v_sb[:, :, D], in_=vones[:])
                        for j in range(8):
                            chunk(val_sb, valT[b], j)
                        for j in range(2, 8):
                            chunk(qry_sb, qryT[b], j)
                if not defer_loads:
                    loads()

                def qk_unit(tmp, packed, w_sb, src, nchi):
                    def emit():
                        psq = psB.tile([D, 512], F32, name="psq", tag="psB")
                        for t in range(4):
                            nc.tensor.matmul(
                                psq, w_sb[:, t, :], src[:, t, ds(nchi * 512, 512)],
                                start=(t == 0), stop=(t == 3),
                            )
                        with nc.allow_low_precision(reason="fp8 scores"):
                            nc.vector.tensor_copy(tmp[:, ds(nchi * 512, 512)], psq)
                        if fp8 and nchi % 2 == 1:  # repack a 1024-col quarter
                            cs = ds((nchi - 1) * 512, 1024)
                            nc.sync.dma_start(out=packed[:, 0, cs], in_=tmp[0:32, cs])
                            nc.sync.dma_start(out=packed[:, 1, cs], in_=tmp[32:64, cs])
                    return emit

                def v_unit(mt):
                    def emit():
                        psv = psB.tile([128, D], F32, name="psv", tag="psB")
                        for t in range(4):
                            nc.tensor.matmul(
                                psv, val_sb[:, t, ts(mt, 128)], wv_sb[:, t, :],
                                start=(t == 0), stop=(t == 3),
                            )
                        nc.vector.tensor_copy(v_sb[:, mt, 0:D], psv)
                    return emit

                k_units = [once(qk_unit(kT_tmp, kT_sb, wk_sb, key_sb, i)) for i in range(8)]
                q_units = [once(qk_unit(qT_tmp, qT_sb, wq_sb, qry_sb, i)) for i in range(8)]
                v_units = [once(v_unit(mt)) for mt in range(32)]
                return loads, qT_sb, kT_sb, v_sb, k_units, q_units, v_units, qT_tmp, kT_tmp

            def proj_units(b):
                """A2A for batch b + unit closures for the output projection."""
                with nc.named_scope(f"proj{b}"):
                    nc.gpsimd.collective_compute(
                        "AllToAll",
                        mybir.AluOpType.bypass,
                        replica_groups=[list(range(NCORES))],
                        ins=[a2a_in[b].opt()],
                        outs=[a2a_out[b].opt()],
                    )
                    xf = []
                    for t in range(4):
                        xft = xtpool.tile([128, NSLICE], BF16, name=f"xf{b}{t}", tag=f"xf{t}")
                        nc.sync.dma_start(out=xft[0:D, :], in_=a2a_out[b][2 * t])
                        nc.sync.dma_start(out=xft[D:128, :], in_=a2a_out[b][2 * t + 1])
                        xf.append(xft)

                def p_unit(ct, hf):
                    def emit():
                        psp = psB.tile([128, 512], F32, name="psp", tag="psB")
                        for et in range(4):
                            nc.tensor.matmul(
                                psp, wp_sb[:, et, ts(ct, 128)], xf[et][:, ts(hf, 512)],
                                start=(et == 0), stop=(et == 3),
                            )
                        out_sb = small.tile([128, 512], F32, name="out_sb", bufs=4)
                        nc.scalar.activation(out_sb, psp, ID_FUNC, bias=bp_sb[:, ct : ct + 1])
                        nc.sync.dma_start(out=out_ext[b][ts(ct, 128), ts(hf, 512)], in_=out_sb)
                    return emit

                return [once(p_unit(ct, hf)) for ct in range(4) for hf in range(NSLICE // 512)]

            def do_attn(b, qkv, own_weave, fillers, fill_from_wci):
                """Attention for batch b, software-pipelined.

                own_weave: weave this batch's own k/q/v units in by deadline
                (first-batch cold start). fillers: unconstrained unit closures
                (next batch's qkv, previous batch's projection) drained one
                per m-tile starting at chunk fill_from_wci.
                """
                loads, qT_sb, kT_sb, v_sb, k_units, q_units, v_units, qT_tmp, kT_tmp = qkv
                for u in k_units + q_units + v_units:
                    if not own_weave:
                        u()  # one-shot: no-op for units already woven elsewhere
                if own_weave:
                    k_units[0]()
                    k_units[1]()
                    q_units[0]()
                    q_units[1]()
                    for mt in range(5):
                        v_units[mt]()

                xT_sb = xtpool.tile([D, N], BF16, name="xT_sb", tag="xT_sb", bufs=1)
                with nc.named_scope(f"attn{b}"):
                    def scores(wci, mt):
                        pss = psS.tile([128, WCH], F32, name="pss", tag="pss")
                        for h in range(2):
                            if fp8:
                                nc.tensor.matmul(
                                    pss[:, ts(h, 512)],
                                    kT_sb[:, :, ts(mt, 128)],
                                    qT_sb[:, :, ds(wci * WCH + h * 512, 512)],
                                    start=True, stop=True,
                                    perf_mode=DR,
                                )
                            else:
                                nc.tensor.matmul(
                                    pss[:, ts(h, 512)],
                                    kT_sb[:, ts(mt, 128)],
                                    qT_sb[:, ds(wci * WCH + h * 512, 512)],
                                    start=True, stop=True,
                                )
                        return pss

                    # software-pipelined: scores(mt+1) is emitted before PV(mt)
                    # so the in-order PE stream never stalls behind the exp.
                    pss_cur = scores(0, 0)
                    for wci in range(N // WCH):
                        pso = [
                            psO.tile([D + 1, 512], F32, name=f"pso{h}", tag=f"pso{h}", bufs=1)
                            for h in range(2)
                        ]
                        for mt in range(32):
                            if own_weave and wci == 0:
                                # cold start: feed k/q/v units just ahead of
                                # use; all own units are done by end of wci 0
                                # so fillers (from wci 1) can't pool-deadlock.
                                if mt % 4 == 0 and mt // 4 + 2 < 8:
                                    k_units[mt // 4 + 2]()
                                if mt + 5 < 32:
                                    v_units[mt + 5]()
                                if mt in (8, 12, 16, 20, 24, 28):
                                    q_units[2 + (mt - 8) // 4]()
                            if wci >= fill_from_wci and fillers:
                                fillers.pop(0)()
                            p_sb = ppool.tile([128, WCH], BF16, name="p_sb")
                            nc.scalar.activation(
                                p_sb[:, 0:XACT], pss_cur[:, 0:XACT], EXP_FUNC, scale=SCALE
                            )
                            with nc.allow_low_precision(reason="schraudolph exp"):
                                nc.vector.tensor_scalar(
                                    p_sb.bitcast(I16)[:, XACT:WCH],
                                    pss_cur[:, XACT:WCH],
                                    SCHRA_A, SCHRA_B,
                                    op0=mybir.AluOpType.mult,
                                    op1=mybir.AluOpType.add,
                                )
                            if mt < 31:
                                pss_cur = scores(wci, mt + 1)
                            elif wci < N // WCH - 1:
                                pss_cur = scores(wci + 1, 0)
                            for h in range(2):
                                nc.tensor.matmul(
                                    pso[h], v_sb[:, mt, :], p_sb[:, ts(h, 512)],
                                    start=(mt == 0), stop=(mt == 31),
                                )
                        # normalize: xT = pso[0:64] / pso[64]. Copy PSUM->SBUF
                        # first so the pso slots free up right away; the rest
                        # of the chain runs off the SBUF copy while the next
                        # chunk's PV matmuls accumulate.
                        o_sb = []
                        for h in range(2):
                            o = small.tile([D + 1, 512], F32, name=f"o_sb{h}", tag=f"o_sb{h}")
                            nc.vector.tensor_copy(o, pso[h])
                            o_sb.append(o)
                        for h in range(2):
                            rinv = small.tile([1, 512], F32R, name="rinv")
                            with nc.allow_low_precision(reason="softmax denom bcast"):
                                nc.vector.reciprocal(rinv, o_sb[h][D : D + 1, :])
                            psr = psB.tile([D, 512], F32, name="psr", tag="psB")
                            nc.tensor.matmul(psr, ones_r, rinv, start=True, stop=True)
                            nc.vector.tensor_mul(
                                xT_sb[:, ds(wci * WCH + h * 512, 512)],
                                o_sb[h][0:D, :], psr,
                            )
                if taps:
                    nc.sync.dma_start(out=tap["xT"][:, ds(b * N, N)], in_=xT_sb)
                    if b == 0:
                        nc.sync.dma_start(out=tap["qT"][:], in_=qT_tmp)
                        nc.sync.dma_start(out=tap["kT"][:], in_=kT_tmp)
                        nc.sync.dma_start(out=tap["v"][:], in_=v_sb)

                # scatter this batch's shards for its A2A
                for j in range(NCORES):
                    nc.sync.dma_start(
                        out=a2a_in[b][j],
                        in_=xT_sb[:, ds((j % 4) * NSLICE, NSLICE)],
                    )

            # Batch 0 weaves its own qkv units in by deadline (cold start);
            # batch 1's loads + qkv units fill batch 0's ACT-bound attention;
            # batch 0's A2A + projection fill batch 1's attention (by chunk 2
            # the A2A has long completed). Only batch 1's A2A + projection
            # remain as the tail.
            qkv0 = qkv_units(0)
            qkv1 = qkv_units(1, defer_loads=True)
            fill0 = [qkv1[0]] + qkv1[4] + qkv1[5] + qkv1[6]
            do_attn(0, qkv0, own_weave=True, fillers=fill0, fill_from_wci=1)
            for u in fill0:
                u()
            pu0 = proj_units(0)
            do_attn(1, qkv1, own_weave=False, fillers=list(pu0), fill_from_wci=2)
            for u in pu0:
                u()
            pu1 = proj_units(1)
            for u in pu1:
                u()

    nc.finalize()
    return nc


_NC_CACHE = {}


def _get_nc():
    if "nc" not in _NC_CACHE:
        _NC_CACHE["nc"] = build_nc()
    return _NC_CACHE["nc"]


def _make_in_maps(query, key, value, Wq, Wk, Wv, Wp, bp):
    bf = ml_dtypes.bfloat16
    shared = {}
    for b in range(B):
        shared[f"queryT{b}"] = np.ascontiguousarray(query[b].T).astype(bf)
        shared[f"keyT{b}"] = np.ascontiguousarray(key[b].T).astype(bf)
        shared[f"valueT{b}"] = np.ascontiguousarray(value[b].T).astype(bf)
    shared["wpT"] = np.ascontiguousarray(Wp.T).astype(bf)
    shared["bp"] = np.ascontiguousarray(bp).astype(np.float32)
    shared["vones"] = np.ones((128, 32), bf)
    shared["onesr"] = np.ones((1, D), np.float32)

    in_maps = []
    for j in range(NCORES):
        m = dict(shared)
        m["wqT"] = np.ascontiguousarray(Wq[j * D : (j + 1) * D, :].T).astype(bf)
        m["wkT"] = np.ascontiguousarray(Wk[j * D : (j + 1) * D, :].T).astype(bf)
        m["wvT"] = np.ascontiguousarray(Wv[j * D : (j + 1) * D, :].T).astype(bf)
        in_maps.append(m)
    return in_maps


def run(inputs, trace=False):
    inputs = {k: np.asarray(v) for k, v in inputs.items()}
    nc = _get_nc()
    in_maps = _make_in_maps(**inputs)
    res = run_bass_kernel_spmd(nc, in_maps, core_ids=list(range(NCORES)), trace=trace)
    full = np.empty((B, N, C), np.float32)
    for s in range(4):
        full[0, s * NSLICE : (s + 1) * NSLICE, :] = res.results[s]["out0"].T
        full[1, s * NSLICE : (s + 1) * NSLICE, :] = res.results[4 + s]["out1"].T
    return full, res


def kernel(**inputs):
    return run(inputs, trace=False)[0]


# revision 71
# speedup vs baseline: 1.0494x; 1.0494x over previous
"""Distributed attention kernel for 8 TRN2 NeuronCores.

Problem: B=2, N=M=4096, C=512, H=8 heads, D=64.
  q/k/v = linear(query/key/value); attn = softmax(q k^T / sqrt(D)); out = proj(attn v) + bias

Sharding: 1 head per core (tensor parallel over heads); each core runs both
batches for its head. After each batch's attention, an 8-core AllToAll swaps
head-shards for token-shards and every core runs the full output projection
for that batch (batch 0's A2A + projection hide under batch 1's attention).
Each core emits out0 (a batch-0 token-slice) and out1 (a batch-1 slice);
the host keeps out0 from cores 0-3 and out1 from cores 4-7.

Layouts (per core, head h = core index):
  inputs staged transposed+bf16 on host: queryT/keyT/valueT (C, N)
  qT, kT: (D=64, N) = WxT.T @ xT            (PE, bf16)
  v:      (M, D) natural + a ones column     (so P@V also yields softmax denom)
  sT:     (m-tile 128, n-chunk 1024) = k q^T (PSUM f32, two 512 matmuls)
  P:      exp(sT / sqrt(D))                  (one wide ACT op; no max-sub:
                                              scores ~ N(0,1), max ~6)
  oT:     (65, 512) += v_aug.T @ P-half      (row 64 = sum_m P = softmax denom)
  xT:     oT[0:64] * (1/denom)               (denom bcast via ones-matmul, f32r)
  A2A_b:  shard j = xT_b[:, 1024*(j%4):...]  (bf16, 1MB/rank, all 8 cores)
  proj:   out_b(c', n) = WpT.T @ xfull + bp  (bias per-partition on ACT)
"""

import numpy as np
import ml_dtypes

import concourse.bass as bass
import concourse.mybir as mybir
import concourse.tile as tile
from concourse import bacc
from concourse.bass import ds, ts
from concourse.bass_utils import run_bass_kernel_spmd

BF16 = mybir.dt.bfloat16
F32 = mybir.dt.float32
F32R = mybir.dt.float32r
FP8 = mybir.dt.float8e4
DR = mybir.MatmulPerfMode.DoubleRow

B, N, M, C, H, D = 2, 4096, 4096, 512, 8, 64
SCALE = D ** -0.5
NCORES = 8
NSLICE = N // 4          # 1024 tokens per core after A2A
WCH = 1024               # scores/exp chunk width
EXP_FUNC = mybir.ActivationFunctionType.Exp
ID_FUNC = mybir.ActivationFunctionType.Identity
I16 = mybir.dt.int16
# DVE takes the top XDVE columns of each 1024-wide exp via a bf16 Schraudolph:
# bf16bits(exp(s/8)) ~= round(s * (0.125*128*log2 e) + (127*128 + 0.75)).
# The sawtooth error (~3% on P) mostly cancels in the softmax ratio; the
# uniform-scale part cancels exactly.
XDVE = 256
XACT = WCH - XDVE
SCHRA_A = float(0.125 * 128 * np.log2(np.e))
SCHRA_B = float(127 * 128 + 0.75)


def build_trivial_nc():
    """Same I/O signature, near-zero work — for differential wall-clock timing."""
    nc = bacc.Bacc("TRN2", target_bir_lowering=False, debug=False, num_devices=NCORES)
    params = []
    for b in range(B):
        params.append(nc.declare_dram_parameter(f"queryT{b}", [C, N], BF16, isOutput=False))
        params.append(nc.declare_dram_parameter(f"keyT{b}", [C, M], BF16, isOutput=False))
        params.append(nc.declare_dram_parameter(f"valueT{b}", [C, M], BF16, isOutput=False))
    nc.declare_dram_parameter("wqT", [C, D], BF16, isOutput=False)
    nc.declare_dram_parameter("wkT", [C, D], BF16, isOutput=False)
    nc.declare_dram_parameter("wvT", [C, D], BF16, isOutput=False)
    nc.declare_dram_parameter("wpT", [C, C], BF16, isOutput=False)
    nc.declare_dram_parameter("bp", [C], F32, isOutput=False)
    outs = [nc.declare_dram_parameter(f"out{b}", [C, NSLICE], F32, isOutput=True) for b in range(B)]
    with tile.TileContext(nc) as tc:
        with tc.tile_pool(name="sb", bufs=1) as sb:
            t = sb.tile([128, 16], F32, name="t")
            nc.sync.dma_start(out=t, in_=params[0][:].bitcast(F32)[0:128, 0:16])
            for b in range(B):
                nc.sync.dma_start(out=outs[b][0:128, 0:16], in_=t)
    nc.finalize()
    return nc


def build_nc(taps=False, fp8=False):
    nc = bacc.Bacc(
        "TRN2", target_bir_lowering=False, debug=False, num_devices=NCORES
    )
    tap = {}
    if taps:
        qk_dt = FP8 if fp8 else BF16
        tap["qT"] = nc.declare_dram_parameter("tap_qT", [D, N], qk_dt, isOutput=True)
        tap["kT"] = nc.declare_dram_parameter("tap_kT", [D, M], qk_dt, isOutput=True)
        tap["v"] = nc.declare_dram_parameter("tap_v", [128, 32, D + 1], BF16, isOutput=True)
        tap["xT"] = nc.declare_dram_parameter("tap_xT", [D, B * N], BF16, isOutput=True)

    # Per-core DRAM parameters. Transposed activations are identical on every
    # core; weight slices are per-head.
    qryT = [nc.declare_dram_parameter(f"queryT{b}", [C, N], BF16, isOutput=False) for b in range(B)]
    keyT = [nc.declare_dram_parameter(f"keyT{b}", [C, M], BF16, isOutput=False) for b in range(B)]
    valT = [nc.declare_dram_parameter(f"valueT{b}", [C, M], BF16, isOutput=False) for b in range(B)]
    wqT = nc.declare_dram_parameter("wqT", [C, D], BF16, isOutput=False)
    wkT = nc.declare_dram_parameter("wkT", [C, D], BF16, isOutput=False)
    wvT = nc.declare_dram_parameter("wvT", [C, D], BF16, isOutput=False)
    wpT = nc.declare_dram_parameter("wpT", [C, C], BF16, isOutput=False)
    bp = nc.declare_dram_parameter("bp", [C], F32, isOutput=False)
    vones = nc.declare_dram_parameter("vones", [128, 32], BF16, isOutput=False)
    onesr = nc.declare_dram_parameter("onesr", [1, D], F32, isOutput=False)
    out_ext = [nc.declare_dram_parameter(f"out{b}", [C, NSLICE], F32, isOutput=True) for b in range(B)]

    with tile.TileContext(nc) as tc:
        with (
            tc.tile_pool(name="consts", bufs=1) as consts,
            tc.tile_pool(name="inputs", bufs=1) as inputs,
            tc.tile_pool(name="qk", bufs=2) as qkpool,
            tc.tile_pool(name="vp", bufs=2) as vpool,
            tc.tile_pool(name="ppool", bufs=3) as ppool,
            tc.tile_pool(name="small", bufs=2) as small,
            tc.tile_pool(name="xt", bufs=1) as xtpool,
            tc.tile_pool(name="psS", bufs=2, space="PSUM") as psS,   # scores, 2 banks/slot
            tc.tile_pool(name="psO", bufs=1, space="PSUM") as psO,   # attention accumulators
            tc.tile_pool(name="psB", bufs=2, space="PSUM") as psB,   # qkv/proj/bcast matmuls
            tc.tile_pool(name="dram", bufs=1, space="DRAM") as dram,
        ):
            # ---- constants / weights ----
            wq_sb = consts.tile([128, 4, D], BF16, name="wq_sb")
            wk_sb = consts.tile([128, 4, D], BF16, name="wk_sb")
            wv_sb = consts.tile([128, 4, D], BF16, name="wv_sb")
            wp_sb = consts.tile([128, 4, C], BF16, name="wp_sb")
            bp_sb = consts.tile([128, 4], F32, name="bp_sb")
            ones_r = consts.tile([1, D], F32, name="ones_r")
            nc.sync.dma_start(out=ones_r, in_=onesr[:])
            nc.sync.dma_start(out=wq_sb, in_=wqT[:].rearrange("(t p) d -> p t d", p=128))
            nc.sync.dma_start(out=wk_sb, in_=wkT[:].rearrange("(t p) d -> p t d", p=128))
            nc.sync.dma_start(out=wv_sb, in_=wvT[:].rearrange("(t p) d -> p t d", p=128))
            nc.sync.dma_start(out=wp_sb, in_=wpT[:].rearrange("(t p) d -> p t d", p=128))
            nc.sync.dma_start(out=bp_sb, in_=bp[:].rearrange("(t p) -> p t", p=128))

            a2a_in = [dram.tile([NCORES, D, NSLICE], BF16, name=f"a2a_in{b}") for b in range(B)]
            a2a_out = [dram.tile([NCORES, D, NSLICE], BF16, name=f"a2a_out{b}") for b in range(B)]

            def once(fn):
                done = [False]
                def wrapper():
                    if not done[0]:
                        done[0] = True
                        fn()
                return wrapper

            def qkv_units(b, defer_loads=False):
                """Allocate tiles; return (loads, k/q/v unit closures).

                Each unit closure emits one PSUM-sized piece of the q/k/v
                projections. The caller weaves them into the attention loop so
                the in-order PE stream never sits on a long block of
                projection matmuls while ACT is idle.
                """
                qry_sb = inputs.tile([128, 4, N], BF16, name="qry_sb")
                key_sb = inputs.tile([128, 4, M], BF16, name="key_sb")
                val_sb = inputs.tile([128, 4, M], BF16, name="val_sb")
                if fp8:
                    qT_tmp = qkpool.tile([D, N], FP8, name="qT_tmp", tag="qT_tmp")
                    kT_tmp = qkpool.tile([D, M], FP8, name="kT_tmp", tag="kT_tmp")
                    # DoubleRow-packed: [p, s, n] = row (s*32+p); q and k use
                    # the same packing so the contraction order is consistent.
                    qT_sb = qkpool.tile([32, 2, N], FP8, name="qT_sb", tag="qT_sb")
                    kT_sb = qkpool.tile([32, 2, M], FP8, name="kT_sb", tag="kT_sb")
                else:
                    qT_tmp = qT_sb = qkpool.tile([D, N], BF16, name="qT_sb", tag="qT_sb")
                    kT_tmp = kT_sb = qkpool.tile([D, M], BF16, name="kT_sb", tag="kT_sb")
                v_sb = vpool.tile([128, 32, D + 1], BF16, name="v_sb")

                @once
                def loads():
                    # column-chunked so the first kT/qT units can start after
                    # ~0.5MB instead of a full 4MB tensor load
                    with nc.named_scope(f"qkv{b}"):
                        def chunk(dst, src, j):
                            nc.sync.dma_start(
                                out=dst[:, :, ds(j * 512, 512)],
                                in_=src[:, ds(j * 512, 512)].rearrange(
                                    "(t p) n -> p t n", p=128
                                ),
                            )
                        chunk(key_sb, keyT[b], 0)
                        chunk(qry_sb, qryT[b], 0)
                        chunk(qry_sb, qryT[b], 1)
                        for j in range(1, 8):
                            chunk(key_sb, keyT[b], j)
                        nc.sync.dma_start(out=v_sb[:, :, D], in_=vones[:])
                        for j in range(8):
                            chunk(val_sb, valT[b], j)
                        for j in range(2, 8):
                            chunk(qry_sb, qryT[b], j)
                if not defer_loads:
                    loads()

                def qk_unit(tmp, packed, w_sb, src, nchi):
                    def emit():
                        psq = psB.tile([D, 512], F32, name="psq", tag="psB")
                        for t in range(4):
                            nc.tensor.matmul(
                                psq, w_sb[:, t, :], src[:, t, ds(nchi * 512, 512)],
                                start=(t == 0), stop=(t == 3),
                            )
                        with nc.allow_low_precision(reason="fp8 scores"):
                            nc.vector.tensor_copy(tmp[:, ds(nchi * 512, 512)], psq)
                        if fp8 and nchi % 2 == 1:  # repack a 1024-col quarter
                            cs = ds((nchi - 1) * 512, 1024)
                            nc.sync.dma_start(out=packed[:, 0, cs], in_=tmp[0:32, cs])
                            nc.sync.dma_start(out=packed[:, 1, cs], in_=tmp[32:64, cs])
                    return emit

                def v_unit(mt):
                    def emit():
                        psv = psB.tile([128, D], F32, name="psv", tag="psB")
                        for t in range(4):
                            nc.tensor.matmul(
                                psv, val_sb[:, t, ts(mt, 128)], wv_sb[:, t, :],
                                start=(t == 0), stop=(t == 3),
                            )
                        nc.vector.tensor_copy(v_sb[:, mt, 0:D], psv)
                    return emit

                k_units = [once(qk_unit(kT_tmp, kT_sb, wk_sb, key_sb, i)) for i in range(8)]
                q_units = [once(qk_unit(qT_tmp, qT_sb, wq_sb, qry_sb, i)) for i in range(8)]
                v_units = [once(v_unit(mt)) for mt in range(32)]
                return loads, qT_sb, kT_sb, v_sb, k_units, q_units, v_units, qT_tmp, kT_tmp

            def proj_units(b):
                """A2A for batch b + unit closures for the output projection."""
                with nc.named_scope(f"proj{b}"):
                    nc.gpsimd.collective_compute(
                        "AllToAll",
                        mybir.AluOpType.bypass,
                        replica_groups=[list(range(NCORES))],
                        ins=[a2a_in[b].opt()],
                        outs=[a2a_out[b].opt()],
                    )
                    xf = []
                    for t in range(4):
                        xft = xtpool.tile([128, NSLICE], BF16, name=f"xf{b}{t}", tag=f"xf{t}")
                        nc.sync.dma_start(out=xft[0:D, :], in_=a2a_out[b][2 * t])
                        nc.sync.dma_start(out=xft[D:128, :], in_=a2a_out[b][2 * t + 1])
                        xf.append(xft)

                def p_unit(ct, hf):
                    def emit():
                        psp = psB.tile([128, 512], F32, name="psp", tag="psB")
                        for et in range(4):
                            nc.tensor.matmul(
                                psp, wp_sb[:, et, ts(ct, 128)], xf[et][:, ts(hf, 512)],
                                start=(et == 0), stop=(et == 3),
                            )
                        out_sb = small.tile([128, 512], F32, name="out_sb", bufs=4)
                        nc.scalar.activation(out_sb, psp, ID_FUNC, bias=bp_sb[:, ct : ct + 1])
                        nc.sync.dma_start(out=out_ext[b][ts(ct, 128), ts(hf, 512)], in_=out_sb)
                    return emit

                return [once(p_unit(ct, hf)) for ct in range(4) for hf in range(NSLICE // 512)]

            def do_attn(b, qkv, own_weave, fillers, fill_from_wci):
                """Attention for batch b, software-pipelined.

                own_weave: weave this batch's own k/q/v units in by deadline
                (first-batch cold start). fillers: unconstrained unit closures
                (next batch's qkv, previous batch's projection) drained one
                per m-tile starting at chunk fill_from_wci.
                """
                loads, qT_sb, kT_sb, v_sb, k_units, q_units, v_units, qT_tmp, kT_tmp = qkv
                for u in k_units + q_units + v_units:
                    if not own_weave:
                        u()  # one-shot: no-op for units already woven elsewhere
                if own_weave:
                    k_units[0]()
                    k_units[1]()
                    q_units[0]()
                    q_units[1]()
                    for mt in range(5):
                        v_units[mt]()

                xT_sb = xtpool.tile([D, N], BF16, name="xT_sb", tag="xT_sb", bufs=1)
                with nc.named_scope(f"attn{b}"):
                    def scores(wci, mt):
                        pss = psS.tile([128, WCH], F32, name="pss", tag="pss")
                        for h in range(2):
                            if fp8:
                                nc.tensor.matmul(
                                    pss[:, ts(h, 512)],
                                    kT_sb[:, :, ts(mt, 128)],
                                    qT_sb[:, :, ds(wci * WCH + h * 512, 512)],
                                    start=True, stop=True,
                                    perf_mode=DR,
                                )
                            else:
                                nc.tensor.matmul(
                                    pss[:, ts(h, 512)],
                                    kT_sb[:, ts(mt, 128)],
                                    qT_sb[:, ds(wci * WCH + h * 512, 512)],
                                    start=True, stop=True,
                                )
                        return pss

                    def make_norm(o_sb, r_sb, wci):
                        # deferred normalize: xT = o_sb * (1/r_sb); woven into
                        # the NEXT chunk's loop so the in-order PE stream
                        # always has queued work ahead of the bcast.
                        rinvs = []

                        def stage_a():
                            for h in range(2):
                                rinv = small.tile([1, 512], F32, name="rinv")
                                nc.vector.reciprocal_approx_fast(rinv, r_sb[h])
                                rinvs.append(rinv)

                        def stage_b():
                            for h in range(2):
                                psr = psB.tile([D, 512], F32, name="psr", tag="psB")
                                nc.tensor.matmul(psr, ones_r, rinvs[h], start=True, stop=True)
                                nc.vector.tensor_mul(
                                    xT_sb[:, ds(wci * WCH + h * 512, 512)],
                                    o_sb[h], psr,
                                )
                        return [stage_a, stage_b]

                    # software-pipelined: scores(mt+1) is emitted before PV(mt)
                    # so the in-order PE stream never stalls behind the exp.
                    pss_cur = scores(0, 0)
                    pending_norm = None
                    for wci in range(N // WCH):
                        pso = [
                            psO.tile([D + 1, 512], F32, name=f"pso{h}", tag=f"pso{h}", bufs=1)
                            for h in range(2)
                        ]
                        for mt in range(32):
                            if pending_norm is not None and mt in (1, 3):
                                pending_norm[(mt - 1) // 2]()
                            if own_weave and wci == 0:
                                # cold start: feed k/q/v units just ahead of
                                # use; all own units are done by end of wci 0
                                # so fillers (from wci 1) can't pool-deadlock.
                                if mt % 4 == 0 and mt // 4 + 2 < 8:
                                    k_units[mt // 4 + 2]()
                                if mt + 5 < 32:
                                    v_units[mt + 5]()
                                if mt in (8, 12, 16, 20, 24, 28):
                                    q_units[2 + (mt - 8) // 4]()
                            if wci >= fill_from_wci and fillers:
                                fillers.pop(0)()
                            p_sb = ppool.tile([128, WCH], BF16, name="p_sb")
                            nc.scalar.activation(
                                p_sb[:, 0:XACT], pss_cur[:, 0:XACT], EXP_FUNC, scale=SCALE
                            )
                            with nc.allow_low_precision(reason="schraudolph exp"):
                                nc.vector.tensor_scalar(
                                    p_sb.bitcast(I16)[:, XACT:WCH],
                                    pss_cur[:, XACT:WCH],
                                    SCHRA_A, SCHRA_B,
                                    op0=mybir.AluOpType.mult,
                                    op1=mybir.AluOpType.add,
                                )
                            if mt < 31:
                                pss_cur = scores(wci, mt + 1)
                            elif wci < N // WCH - 1:
                                pss_cur = scores(wci + 1, 0)
                            for h in range(2):
                                nc.tensor.matmul(
                                    pso[h], v_sb[:, mt, :], p_sb[:, ts(h, 512)],
                                    start=(mt == 0), stop=(mt == 31),
                                )
        # free the pso slots promptly via PSUM->SBUF copies;
                        # the rest of the normalize is deferred into the next
                        # chunk (or emitted now for the last chunk). The denom
                        # row is copied to a partition-0 tile because the
                        # custom-DVE reciprocal can't take partition offsets.
                        o_sb, r_sb = [], []
                        for h in range(2):
                            o = small.tile([D, 512], F32, name=f"o_sb{h}", tag=f"o_sb{h}")
                            nc.vector.tensor_copy(o, pso[h][0:D, :])
                            r = small.tile([1, 512], F32, name=f"r_sb{h}", tag=f"r_sb{h}")
                            nc.vector.tensor_copy(r, pso[h][D : D + 1, :])
                            o_sb.append(o)
                            r_sb.append(r)
                        pending_norm = make_norm(o_sb, r_sb, wci)
                        if wci == N // WCH - 1:
                            pending_norm[0]()
                            pending_norm[1]()
                            pending_norm = None
                if taps:
                    nc.sync.dma_start(out=tap["xT"][:, ds(b * N, N)], in_=xT_sb)
                    if b == 0:
                        nc.sync.dma_start(out=tap["qT"][:], in_=qT_tmp)
                        nc.sync.dma_start(out=tap["kT"][:], in_=kT_tmp)
                        nc.sync.dma_start(out=tap["v"][:], in_=v_sb)

                # scatter this batch's shards for its A2A
                for j in range(NCORES):
                    nc.sync.dma_start(
                        out=a2a_in[b][j],
                        in_=xT_sb[:, ds((j % 4) * NSLICE, NSLICE)],
                    )

            # Batch 0 weaves its own qkv units in by deadline (cold start);
            # batch 1's loads + qkv units fill batch 0's ACT-bound attention;
            # batch 0's A2A + projection fill batch 1's attention (by chunk 2
            # the A2A has long completed). Only batch 1's A2A + projection
            # remain as the tail.
            qkv0 = qkv_units(0)
            qkv1 = qkv_units(1, defer_loads=True)
            fill0 = [qkv1[0]] + qkv1[4] + qkv1[5] + qkv1[6]
            do_attn(0, qkv0, own_weave=True, fillers=fill0, fill_from_wci=1)
            for u in fill0:
                u()
            pu0 = proj_units(0)
            do_attn(1, qkv1, own_weave=False, fillers=list(pu0), fill_from_wci=2)
            for u in pu0:
                u()
            pu1 = proj_units(1)
            for u in pu1:
                u()

    nc.finalize()
    return nc


_NC_CACHE = {}


def _get_nc():
    if "nc" not in _NC_CACHE:
        _NC_CACHE["nc"] = build_nc()
    return _NC_CACHE["nc"]


def _make_in_maps(query, key, value, Wq, Wk, Wv, Wp, bp):
    bf = ml_dtypes.bfloat16
    shared = {}
    for b in range(B):
        shared[f"queryT{b}"] = np.ascontiguousarray(query[b].T).astype(bf)
        shared[f"keyT{b}"] = np.ascontiguousarray(key[b].T).astype(bf)
        shared[f"valueT{b}"] = np.ascontiguousarray(value[b].T).astype(bf)
    shared["wpT"] = np.ascontiguousarray(Wp.T).astype(bf)
    shared["bp"] = np.ascontiguousarray(bp).astype(np.float32)
    shared["vones"] = np.ones((128, 32), bf)
    shared["onesr"] = np.ones((1, D), np.float32)

    in_maps = []
    for j in range(NCORES):
        m = dict(shared)
        m["wqT"] = np.ascontiguousarray(Wq[j * D : (j + 1) * D, :].T).astype(bf)
        m["wkT"] = np.ascontiguousarray(Wk[j * D : (j + 1) * D, :].T).astype(bf)
        m["wvT"] = np.ascontiguousarray(Wv[j * D : (j + 1) * D, :].T).astype(bf)
        in_maps.append(m)
    return in_maps


def run(inputs, trace=False):
    inputs = {k: np.asarray(v) for k, v in inputs.items()}
    nc = _get_nc()
    in_maps = _make_in_maps(**inputs)
    res = run_bass_kernel_spmd(nc, in_maps, core_ids=list(range(NCORES)), trace=trace)
    full = np.empty((B, N, C), np.float32)
    for s in range(4):
        full[0, s * NSLICE : (s + 1) * NSLICE, :] = res.results[s]["out0"].T
        full[1, s * NSLICE : (s + 1) * NSLICE, :] = res.results[4 + s]["out1"].T
    return full, res


def kernel(**inputs):
    return run(inputs, trace=False)[0]
